# revision 42
# baseline (speedup 1.0000x reference)
"""Grouped MoE dispatcher kernel for 8 Trainium2 NeuronCores.

Expert-parallel: 8 experts per core. Host performs the dispatch (stable sort
of (token, slot) assignments by expert id — identical to the reference's
fixed-capacity grouped dispatch) and supplies each core its 8 experts'
tokens pre-gathered and pre-tiled into SBUF layout; the device runs the
grouped FFN (x@W1 -> silu -> @W2, scaled by routing weight) as bf16 matmuls
with fp32 PSUM accumulation; host scatter-combines the two slots per token.

All DRAM tensors are laid out so every DMA descriptor is one full SBUF
partition line (1-8 KB contiguous), and expert 0's x/W1 are additionally
split into fine-grained chunks so the first matmul starts as soon as
~256 KB have landed. y is stored bf16 (host upcasts when combining).

Problem constants (hardcoded): B=16384 tokens, K=2, E=64 experts, H=512,
F=1024; I/O fp32, matmul operands bf16 (end-to-end rel err ~3.4e-3).
"""

import json
import os

import ml_dtypes
import numpy as np

import concourse.bass as bass
import concourse.bass2jax as bass2jax
import concourse.bass_utils as bass_utils
import concourse.mybir as mybir
import concourse.tile as tile_mod
from concourse.tile import TileContext, ScopedClock
from concourse.bass_utils import run_bass_kernel_spmd

B = 16384
K = 2
E = 64
H = 512
F = 1024
NCORES = 8
EPC = E // NCORES          # experts per core = 8
N = B * K                  # assignments = 32768
CAP = N // E               # per-expert capacity = 512
TPC = EPC * CAP            # tokens (assignments) per core = 4096
P = 128                    # partitions

FP32 = mybir.dt.float32
BF16 = mybir.dt.bfloat16

HS = H // P   # 4 contraction subtiles for stage 1
FS = F // P   # 8 F subtiles (stage-1 out partitions / stage-2 contraction)
CS = CAP // P  # 4 token subtiles per expert


# ---------------------------------------------------------------------------
# Workaround: the walrus build in this container rejects instructions carrying
# more than one sync-wait ("Too many sync wait commands", CoreV3GenImpl
# setupSyncWait), while Tile routinely attaches several waits to one
# instruction. Post-process the BIR JSON before compilation: move extra waits
# onto single-wait NoOps inserted immediately before the instruction on the
# same (in-order) engine sequencer — a strictly stronger ordering, so always
# semantics-preserving.
# ---------------------------------------------------------------------------

_MAX_WAITS = 1


def _split_multi_waits(bir: dict) -> dict:
    ctr = 0
    for fn in bir.get("functions", []):
        for bb in fn.get("blocks", []):
            out = []
            for ins in bb.get("instructions", []):
                si = ins.get("sync_info")
                ow = (si or {}).get("on_wait") or []
                if len(ow) > _MAX_WAITS:
                    for w in ow[: -_MAX_WAITS]:
                        ctr += 1
                        out.append(
                            {
                                "debug": ins.get("debug"),
                                "engine": ins.get("engine"),
                                "ins": [],
                                "name": f"I-WSPLIT-{ctr}",
                                "opcode": "NoOp",
                                "outs": [],
                                "sync_info": {"on_update": [], "on_wait": [w]},
                            }
                        )
                    si["on_wait"] = ow[-_MAX_WAITS:]
                out.append(ins)
            bb["instructions"] = out
    return bir


_orig_compile_bir_kernel = bass_utils.compile_bir_kernel


def _compile_bir_kernel_split(bir_json, tmpdir, neff_name="file.neff"):
    bir = json.loads(bir_json)
    bir = _split_multi_waits(bir)
    return _orig_compile_bir_kernel(json.dumps(bir).encode(), tmpdir, neff_name)


if bass_utils.compile_bir_kernel is not _compile_bir_kernel_split:
    bass_utils.compile_bir_kernel = _compile_bir_kernel_split
    bass2jax.compile_bir_kernel = _compile_bir_kernel_split


def _cheap_drain_and_barrier(self, tick_clock, wait_clock):
    # Cheap kernel tail: stock TileContext runs drain + two all-engine
    # butterfly barriers around the semaphore clear (~8us). Instead, attach
    # every outstanding proc's final tick as waits on GpSimd — the engine
    # that performs the DGE/sem clear. Once those waits pass, every engine
    # is quiescent, so the clear is safe and the other engines simply halt.
    # (The multi-wait NOP is split into single-wait NOPs by the BIR pass.)
    nc = self.nc
    collector = nc.gpsimd.nop(nofuse=True)
    wait_clock.add_sem_waits(
        collector.ins, ScopedClock({None: tick_clock.global_clock})
    )
    nc.sync.drain()
    assert self.sems is not None
    popped = nc._tile_sem_poison_stack.pop()
    assert popped is self._sem_poison
    nc.clear_and_free_semaphores(list(self.sems.allocated().values()))


tile_mod.TileContext._drain_and_barrier = _cheap_drain_and_barrier


def _build_bass(cdt=BF16):
    nc = bass.Bass(trn_type="TRN2")
    # Pre-tiled inputs: every tensor is laid out so one SBUF partition line
    # is one contiguous DRAM run (big DMA descriptors).
    #   xd[e, p, c, t]      = x_sorted[e*CAP + t, c*128 + p]
    #   w1e0[f, p, c, fi]   = W1[core_e0, c*128 + p, f*128 + fi]   (expert 0)
    #   w1d[e, h2, p, c, fh]= W1[e, c*128 + p, h2*F/2 + fh]        (e >= 1)
    #   w2d[e, p, s, h]     = W2[e, s*128 + p, h]
    xd = nc.dram_tensor("xd", [EPC, P, HS, CAP], cdt, kind="ExternalInput")
    w1d = nc.dram_tensor(
        "w1d", [EPC, 2, P, HS, F // 2], cdt, kind="ExternalInput"
    )
    # expert 0's W1 again, as four quarter tiles (2KB lines) so the critical
    # first fill can be balanced across all three queues against each
    # f-group's deadline
    w1q = nc.dram_tensor("w1q", [4, P, HS, F // 4], cdt, kind="ExternalInput")
    w2d = nc.dram_tensor("w2d", [EPC, P, FS, H], cdt, kind="ExternalInput")
    wt = nc.dram_tensor("wt", [P, TPC // P], FP32, kind="ExternalInput")
    y = nc.dram_tensor("y", [TPC, H], cdt, kind="ExternalOutput")

    # Raw (uninitialized) SBUF scratch for the HAM warm-up matmuls: contents
    # are irrelevant (the scratch PSUM is never read), and skipping the
    # memset lets the first warm-up issue the moment the engine preamble
    # ends (~7.4us) instead of waiting on a memset + semaphore (~8.6us).
    warm_lhs = nc.alloc_sbuf_tensor("warm_lhs", [P, P], cdt).ap()
    warm_rhs = nc.alloc_sbuf_tensor("warm_rhs", [P, CAP], cdt).ap()

    with TileContext(nc) as tc:
        with (
            tc.tile_pool(name="weights", bufs=3) as wpool,
            tc.tile_pool(name="acts", bufs=3) as apool,
            tc.tile_pool(name="outs", bufs=8) as opool,
            tc.tile_pool(name="consts", bufs=1) as cpool,
            tc.tile_pool(name="psum", bufs=4, space="PSUM") as pspool,
        ):
            wt_t = cpool.tile([P, TPC // P], FP32, tag="wt")

            # HAM warm-up: PE runs at 1.2 GHz until ~3.4us of sustained
            # activity, and any PE idle gap resets the busy window. 11 dummy
            # N=512 matmuls on uninitialized scratch span the window between
            # the engine preamble end (~7.4us) and the measured arrival of
            # expert 0's x + first W1 quarter (~12.3us: queue wake latency
            # + ~115GB/s per queue), so the PE is continuously busy and
            # every real matmul runs at 2.4 GHz from the start.
            # Deliberately overshoot past data-ready (~12.7-13.3us): an
            # undershoot leaves a PE idle gap that voids the HAM busy
            # window (~3us of cold real matmuls); an overshoot only delays
            # the first real matmul by the overlap.
            warm_ps = pspool.tile([P, H], FP32, tag="ps2")
            for _ in range(14):
                nc.tensor.matmul(
                    warm_ps[:], warm_lhs, warm_rhs, start=True, stop=True
                )

            x_tiles = {}
            hid_tiles = {}
            w2_tiles = {}
            w1_tiles = {}

            # Three HWDGE rings (only SP/ACT/POOL can initiate DMA). Each
            # ring drains through ONE hw queue at a near-constant
            # ~115-150 GB/s, so sustained load is balanced across all
            # three and the critical first MB (x(0) + first W1 quarter)
            # is split three ways:
            #   scalar(ACT): x (expert 0: first c-half) + w2 first halves
            #   sync(SP):    x(0) second c-half, w1(0) second half,
            #                then w1a + w2 second halves
            #   gpsimd(POOL): w1(0) first-half quarters + wt, then w1b
            #                 and the y stores (bf16, light)
            def load_x(e):
                if e == 0:
                    xa = cpool.tile([P, 2, CAP], cdt, tag="x0a")
                    nc.scalar.dma_start(xa[:], xd[0, :, 0:2, :])
                    xb = cpool.tile([P, 2, CAP], cdt, tag="x0b")
                    nc.sync.dma_start(xb[:], xd[0, :, 2:4, :])
                    x_tiles[e] = (xa, xb)
                else:
                    x_t = apool.tile([P, HS, CAP], cdt, tag="x")
                    nc.scalar.dma_start(x_t[:], xd[e])
                    x_tiles[e] = x_t

            def load_w1(e):
                if e == 0:
                    # four quarter tiles, one per pair of f-groups, placed
                    # so each lands before its group's deadline: q0/q1/q3
                    # on gpsimd, q2 on sync (behind x0b), wt last
                    qs = []
                    for qi, eng in [
                        (0, nc.gpsimd),
                        (1, nc.gpsimd),
                        (2, nc.sync),
                        (3, nc.gpsimd),
                    ]:
                        q_t = cpool.tile(
                            [P, HS, F // 4], cdt, tag=f"w1q{qi}"
                        )
                        eng.dma_start(q_t[:], w1q[qi])
                        qs.append(q_t)
                    # routing weights ride the gpsimd queue, off the
                    # critical fill path
                    nc.gpsimd.dma_start(wt_t[:], wt[:])
                    w1_tiles[e] = qs
                else:
                    # halves on separate queues; the first FS/2 matmul
                    # groups only need w1a
                    w1a_t = wpool.tile([P, HS, F // 2], cdt, tag="w1a")
                    nc.sync.dma_start(w1a_t[:], w1d[e, 0])
                    w1b_t = wpool.tile([P, HS, F // 2], cdt, tag="w1b")
                    nc.gpsimd.dma_start(w1b_t[:], w1d[e, 1])
                    w1_tiles[e] = (w1a_t, w1b_t)

            def load_w2(e):
                # halves on the scalar/sync queues, issued after
                # x(e+1)/w1a(e+1): w2(e) isn't needed until stage2(e) and
                # must never delay stage-1 data (the fill is queue-bound)
                w2a_t = wpool.tile([P, FS // 2, H], cdt, tag="w2a")
                nc.scalar.dma_start(w2a_t[:], w2d[e, :, 0 : FS // 2, :])
                w2b_t = wpool.tile([P, FS // 2, H], cdt, tag="w2b")
                nc.sync.dma_start(w2b_t[:], w2d[e, :, FS // 2 :, :])
                w2_tiles[e] = (w2a_t, w2b_t)

            def stage1(e):
                x_src = x_tiles.pop(e)
                w1_src = w1_tiles.pop(e)
                # ---- stage 1: hid[F, tok] = silu(W1^T x) ----
                hid_t = apool.tile([P, FS, CAP], cdt, tag="hid")
                hid_tiles[e] = hid_t
                for f in range(FS):
                    ps1 = pspool.tile([P, CAP], FP32, tag="ps1")
                    for c in range(HS):
                        if e == 0:
                            w1h = w1_src[f // 2]
                            fh = f % 2
                            rhs = x_src[c // 2][:, c % 2, :]
                        else:
                            w1h = w1_src[f // (FS // 2)]
                            fh = f % (FS // 2)
                            rhs = x_src[:, c, :]
                        nc.tensor.matmul(
                            ps1[:],
                            w1h[:, c, fh * P : (fh + 1) * P],
                            rhs,
                            start=(c == 0),
                            stop=(c == HS - 1),
                        )
                    nc.scalar.activation(
                        hid_t[:, f, :], ps1[:], mybir.ActivationFunctionType.Silu
                    )

            def stage2(e):
                # ---- stage 2: y[tok, H] = (hid^T W2) * wt, stored bf16 ----
                hid_t = hid_tiles.pop(e)
                w2a_t, w2b_t = w2_tiles.pop(e)
                for j in range(CS):
                    gj = e * CS + j  # global token-chunk index within this core
                    ps2 = pspool.tile([P, H], FP32, tag="ps2")
                    for f in range(FS):
                        w2h = w2a_t if f < FS // 2 else w2b_t
                        nc.tensor.matmul(
                            ps2[:],
                            hid_t[:, f, j * P : (j + 1) * P],
                            w2h[:, f % (FS // 2), :],
                            start=(f == 0),
                            stop=(f == FS - 1),
                        )
                    y_t = opool.tile([P, H], cdt, tag="y")
                    nc.vector.tensor_scalar_mul(
                        y_t[:], ps2[:], wt_t[:, gj : gj + 1]
                    )
                    base = e * CAP + j * P
                    if e == EPC - 1 and j == CS - 1:
                        # final store split across the two idle, empty
                        # scalar/sync queues (gpsimd still drains j<3's
                        # stores — a backlog there would gate the drain)
                        nc.scalar.dma_start(
                            y[base : base + P // 2, :], y_t[0 : P // 2, :]
                        )
                        nc.sync.dma_start(
                            y[base + P // 2 : base + P, :], y_t[P // 2 :, :]
                        )
                    else:
                        nc.gpsimd.dma_start(y[base : base + P, :], y_t[:])

            # Software pipeline: stage2(e) is issued after stage1(e+1) so the
            # PE never waits on the ACT (silu) tail of its own expert; loads
            # run one expert ahead of compute.
            load_x(0)
            load_w1(0)
            for e in range(EPC):
                if e + 1 < EPC:
                    load_x(e + 1)
                    load_w1(e + 1)
                load_w2(e)
                stage1(e)
                if e > 0:
                    stage2(e - 1)
            stage2(EPC - 1)
    return nc


_NC_CACHE = {}


def _get_bass(cdt):
    if cdt not in _NC_CACHE:
        _NC_CACHE[cdt] = _build_bass(cdt)
    return _NC_CACHE[cdt]


def kernel(hidden_states, expert_weights, expert_ids, W1, W2):
    hidden_states = np.ascontiguousarray(hidden_states, dtype=np.float32)
    expert_weights = np.ascontiguousarray(expert_weights, dtype=np.float32)
    expert_ids = np.ascontiguousarray(expert_ids, dtype=np.int32)
    W1 = np.ascontiguousarray(W1, dtype=np.float32)
    W2 = np.ascontiguousarray(W2, dtype=np.float32)

    # Dispatch: stable sort of flattened (token, slot) assignments by expert
    # id; fixed-capacity groups of CAP rows, exactly as the reference does.
    flat_ids = expert_ids.reshape(-1)
    order = np.argsort(flat_ids, kind="stable")
    tok = order // K
    w_sorted = expert_weights.reshape(-1)[order]

    np_cdt = ml_dtypes.bfloat16
    xg = hidden_states.astype(np_cdt)[tok]  # [N, H] sorted-assignment rows
    W1_c = W1.astype(np_cdt)
    W2_c = W2.astype(np_cdt)

    in_maps = []
    for core in range(NCORES):
        sl = slice(core * TPC, (core + 1) * TPC)
        g0 = core * EPC
        # x: [e, p, c, t] = xg[core_tok + e*CAP + t, c*128 + p]
        xd = np.ascontiguousarray(
            xg[sl].reshape(EPC, CAP, HS, P).transpose(0, 3, 2, 1)
        )
        # w1 halves: [e, h2, p, c, fh] = W1[g0+e, c*128+p, h2*512+fh]
        w1d = np.ascontiguousarray(
            W1_c[g0 : g0 + EPC]
            .reshape(EPC, HS, P, 2, F // 2)
            .transpose(0, 3, 2, 1, 4)
        )
        # expert 0 quarters: [q, p, c, fq] = W1[g0, c*128+p, q*256+fq]
        w1qa = np.ascontiguousarray(
            W1_c[g0].reshape(HS, P, 4, F // 4).transpose(2, 1, 0, 3)
        )
        # w2: [e, p, s, h] = W2[g0+e, s*128+p, h]
        w2d = np.ascontiguousarray(
            W2_c[g0 : g0 + EPC].reshape(EPC, FS, P, H).transpose(0, 2, 1, 3)
        )
        in_maps.append(
            {
                "xd": xd,
                "w1d": w1d,
                "w1q": w1qa,
                "w2d": w2d,
                "wt": np.ascontiguousarray(
                    w_sorted[sl].reshape(TPC // P, P).T
                ),
            }
        )

    nc = _get_bass(BF16)
    res = run_bass_kernel_spmd(nc, in_maps, core_ids=list(range(NCORES)))
    global _LAST_RESULTS
    _LAST_RESULTS = res
    y_all = np.concatenate(
        [r["y"].astype(np.float32) for r in res.results], axis=0
    )  # [N, H]

    # Combine: undo the sort, then sum each token's K weighted slot outputs.
    y_unsorted = np.empty_like(y_all)
    y_unsorted[order] = y_all
    out = y_unsorted.reshape(B, K, H).sum(axis=1)
    return np.ascontiguousarray(out, dtype=np.float32)
